# revision 1
# baseline (speedup 1.0000x reference)
"""Trainium2 Bass kernel for nn_ComplexNN (3-layer MLP, blended tanh act).

  h1 = blend_act(x @ W1 + b1);  blend_act(z) = z>0 ? 0.9z+0.1tanh(z) : 0.5tanh(z)
  h2 = relu(h1 @ W2 + b2)
  out = h2 @ W3 + b3

Data-parallel over 8 NeuronCores: each core takes 4096 rows of x, weights
replicated. Fully fused on-chip; matmuls in bf16 with fp32 PSUM accumulate.

Layout: activations are kept feature-on-partitions (h1^T, h2^T) so each
matmul's contraction dim lands on partitions with no intermediate
transposes. x is cast fp32->bf16 via SWDGE DMA (DRAM->DRAM, k-major
slices) then DMA-xbar-transposed (DRAM->SBUF). The final layer keeps the
transposed orientation: out^T [10, 4096] goes to DRAM and the host
transposes during the unshard/gather step.

blend_act decomposition (t = tanh(z)):
  blend(z) = 0.9*relu(z) + 0.1*t + 0.4*min(t, 0)
ACT: t = Tanh(psum + b1);  a = Relu(0.9*psum + 0.9*b1)
DVE: m = (t min 0)*0.4 ;  u = 0.1*t + a (STT);  h1 = u + m
"""

import sys

sys.path.insert(0, "/opt/trn_rl_repo")

import ml_dtypes
import numpy as np

import concourse.bass as bass
import concourse.mybir as mybir
import concourse.tile as tile
from concourse import bacc
from concourse.bass_utils import run_bass_kernel_spmd

N_CORES = 8
B, D, H, H2, C = 32768, 512, 1024, 512, 10
BL = B // N_CORES  # rows per core = 4096
# Batch chunk sizes: small first chunks fill the cast->xbar->matmul pipeline
# quickly; later chunks are wide to amortize fixed per-instruction costs; a
# small final chunk shortens the mm2->mm3->store drain tail.
CHUNKS = [256, 256, 512, 1024, 1024, 768, 256]
assert sum(CHUNKS) == BL
KD = D // 128      # 4  k-tiles for mm1
KH = H // 128      # 8  k-tiles for mm2 / h-tiles of h1
KH2 = H2 // 128    # 4  k-tiles for mm3 / h2-tiles of h2

F32 = mybir.dt.float32
BF16 = mybir.dt.bfloat16
AF = mybir.ActivationFunctionType
ALU = mybir.AluOpType


def _body(ctx, tc, outs, ins):
    nc = tc.nc
    x, w1, w2, w3, b1c, b1s, b2c, b3c = ins
    (outT,) = outs

    wpool = ctx.enter_context(tc.tile_pool(name="weights", bufs=1))
    xpool = ctx.enter_context(tc.tile_pool(name="xT", bufs=2 * KD))
    h1pool = ctx.enter_context(tc.tile_pool(name="h1T", bufs=2 * KH))
    h2pool = ctx.enter_context(tc.tile_pool(name="h2T", bufs=2 * KH2))
    tpool = ctx.enter_context(tc.tile_pool(name="tmp", bufs=3))
    opool = ctx.enter_context(tc.tile_pool(name="ostage", bufs=2))
    mmpool = ctx.enter_context(tc.tile_pool(name="mm", bufs=3, space="PSUM"))
    mm3pool = ctx.enter_context(tc.tile_pool(name="mm3", bufs=1, space="PSUM"))
    xbd = ctx.enter_context(tc.tile_pool(name="xbd", bufs=2, space="DRAM"))

    # resident weights / biases (scalar-engine HWDGE queue, so the sync
    # queue is free for the xbar transposes)
    w1s = wpool.tile([128, KD * H], BF16)     # w1s[p, k*H + h]  = W1[k*128+p, h]
    w2s = wpool.tile([128, KH * H2], BF16)    # w2s[p, k*H2 + m] = W2[k*128+p, m]
    w3s = wpool.tile([128, KH2 * C], BF16)    # w3s[p, k*C + c]  = W3[k*128+p, c]
    b1cs = wpool.tile([128, KH], F32)         # b1cs[p, i] = b1[i*128+p]
    b1ss = wpool.tile([128, KH], F32)         # 0.9 * b1
    b2cs = wpool.tile([128, KH2], F32)
    b3cs = wpool.tile([C, 1], F32)            # b3 as per-partition column
    # Weight loads are interleaved into the SWDGE queue AFTER the first
    # chunk's cast (see below): the single SWDGE queue is the ingest
    # critical path, and the first xbar transpose can only start once the
    # first cast transfer completes.
    def load_weights():
        nc.gpsimd.dma_start(out=w1s[:], in_=w1[:])
        nc.gpsimd.dma_start(out=b1cs[:], in_=b1c[:])
        nc.gpsimd.dma_start(out=b1ss[:], in_=b1s[:])
        nc.gpsimd.dma_start(out=w2s[:], in_=w2[:])
        nc.gpsimd.dma_start(out=b2cs[:], in_=b2c[:])
        nc.gpsimd.dma_start(out=w3s[:], in_=w3[:])
        nc.gpsimd.dma_start(out=b3cs[:], in_=b3c[:])

    row0 = 0
    for c, NB in enumerate(CHUNKS):
        bs_cols = [slice(s, min(s + 512, NB)) for s in range(0, NB, 512)]
        rows = slice(row0, row0 + NB)
        row0 += NB

        # contiguous fp32->bf16 cast of the whole chunk (SWDGE DRAM->DRAM),
        # then per-k-slice xbar transposes into SBUF, spread over both
        # HWDGE queues (sync + scalar).
        xb = xbd.tile([max(CHUNKS), D], BF16, tag="xb", name="xb")[:NB]
        nc.gpsimd.dma_start(out=xb[:], in_=x[rows, :])
        if c == 0:
            load_weights()
        xT = []
        for k in range(KD):
            xt = xpool.tile([128, max(CHUNKS)], BF16, tag="xt", name="xt")[:, :NB]
            nc.sync.dma_start(out=xt[:], in_=xb[:, k * 128 : (k + 1) * 128], transpose=True)
            xT.append(xt)

        # ---- mm1 + blend_act ----  h1T[i] [128 h, NB b]
        h1T = []
        for i in range(KH):
            ps = mmpool.tile([128, NB], F32, tag="ps")
            for cols in bs_cols:
                for k in range(KD):
                    nc.tensor.matmul(
                        ps[:, cols],
                        w1s[:, k * H + i * 128 : k * H + (i + 1) * 128],
                        xT[k][:, cols],
                        start=(k == 0),
                        stop=(k == KD - 1),
                    )
            t = tpool.tile([128, NB], BF16, tag="t")
            a = tpool.tile([128, NB], BF16, tag="a")
            nc.scalar.activation(t[:], ps[:], AF.Tanh, bias=b1cs[:, i : i + 1], scale=1.0)
            nc.scalar.activation(a[:], ps[:], AF.Relu, bias=b1ss[:, i : i + 1], scale=0.9)
            m = tpool.tile([128, NB], BF16, tag="m")
            u = tpool.tile([128, NB], BF16, tag="u")
            nc.vector.tensor_scalar(m[:], t[:], 0.0, 0.4, ALU.min, ALU.mult)
            nc.vector.scalar_tensor_tensor(u[:], t[:], 0.1, a[:], ALU.mult, ALU.add)
            h1 = h1pool.tile([128, NB], BF16, tag="h1")
            nc.vector.tensor_add(h1[:], u[:], m[:])
            h1T.append(h1)

        # ---- mm2 + relu (relu on DVE: ACT is the contended engine) ----
        h2T = []
        for j in range(KH2):
            ps2 = mmpool.tile([128, NB], F32, tag="ps")
            for cols in bs_cols:
                for k in range(KH):
                    nc.tensor.matmul(
                        ps2[:, cols],
                        w2s[:, k * H2 + j * 128 : k * H2 + (j + 1) * 128],
                        h1T[k][:, cols],
                        start=(k == 0),
                        stop=(k == KH - 1),
                    )
            h2 = h2pool.tile([128, NB], BF16, tag="h2")
            nc.scalar.activation(h2[:], ps2[:], AF.Relu, bias=b2cs[:, j : j + 1], scale=1.0)
            h2T.append(h2)

        # ---- mm3: out^T [10, NB] = W3^T @ h2 + b3 ----
        ps3 = mm3pool.tile([C, NB], F32, tag="ps3")
        for cols in bs_cols:
            for k in range(KH2):
                nc.tensor.matmul(
                    ps3[:, cols],
                    w3s[:, k * C : (k + 1) * C],
                    h2T[k][:, cols],
                    start=(k == 0),
                    stop=(k == KH2 - 1),
                )
        stage = opool.tile([C, NB], F32, tag="stage")
        nc.vector.tensor_scalar_add(stage[:], ps3[:], b3cs[:])
        # SWDGE (gpsimd) for the store: keeps the HWDGE rings transpose-only,
        # avoiding the DMATranspose<->DMACopy xbar-mode serialization.
        nc.gpsimd.dma_start(out=outT[:, rows], in_=stage[:])


_CACHED = None


def _build():
    global _CACHED
    if _CACHED is not None:
        return _CACHED
    nc = bacc.Bacc(
        "TRN2",
        target_bir_lowering=False,
        debug=False,
        enable_asserts=False,
        num_devices=N_CORES,
    )
    x = nc.dram_tensor("x", [BL, D], F32, kind="ExternalInput").ap()
    w1 = nc.dram_tensor("w1", [128, KD * H], BF16, kind="ExternalInput").ap()
    w2 = nc.dram_tensor("w2", [128, KH * H2], BF16, kind="ExternalInput").ap()
    w3 = nc.dram_tensor("w3", [128, KH2 * C], BF16, kind="ExternalInput").ap()
    b1c = nc.dram_tensor("b1c", [128, KH], F32, kind="ExternalInput").ap()
    b1s = nc.dram_tensor("b1s", [128, KH], F32, kind="ExternalInput").ap()
    b2c = nc.dram_tensor("b2c", [128, KH2], F32, kind="ExternalInput").ap()
    b3c = nc.dram_tensor("b3c", [C, 1], F32, kind="ExternalInput").ap()
    outT = nc.dram_tensor("outT", [C, BL], F32, kind="ExternalOutput").ap()

    from contextlib import ExitStack

    with tile.TileContext(nc) as tc, ExitStack() as ctx:
        _body(ctx, tc, [outT], [x, w1, w2, w3, b1c, b1s, b2c, b3c])
    nc.compile()
    _CACHED = nc
    return nc


def _prep_weights(W1, b1, W2, b2, W3, b3):
    bf = ml_dtypes.bfloat16
    w1h = np.ascontiguousarray(
        W1.astype(bf).reshape(KD, 128, H).transpose(1, 0, 2).reshape(128, KD * H)
    )
    w2h = np.ascontiguousarray(
        W2.astype(bf).reshape(KH, 128, H2).transpose(1, 0, 2).reshape(128, KH * H2)
    )
    w3h = np.ascontiguousarray(
        W3.astype(bf).reshape(KH2, 128, C).transpose(1, 0, 2).reshape(128, KH2 * C)
    )
    b1f = b1.astype(np.float32)
    b1ch = np.ascontiguousarray(b1f.reshape(KH, 128).T)
    b1sh = np.ascontiguousarray((0.9 * b1f).reshape(KH, 128).T)
    b2ch = np.ascontiguousarray(b2.astype(np.float32).reshape(KH2, 128).T)
    b3ch = np.ascontiguousarray(b3.astype(np.float32).reshape(C, 1))
    return w1h, w2h, w3h, b1ch, b1sh, b2ch, b3ch


def _make_in_maps(x, W1, b1, W2, b2, W3, b3):
    x = np.asarray(x, dtype=np.float32)
    w1h, w2h, w3h, b1ch, b1sh, b2ch, b3ch = _prep_weights(
        np.asarray(W1), np.asarray(b1), np.asarray(W2), np.asarray(b2),
        np.asarray(W3), np.asarray(b3),
    )
    return [
        {
            "x": np.ascontiguousarray(x[i * BL : (i + 1) * BL]),
            "w1": w1h, "w2": w2h, "w3": w3h,
            "b1c": b1ch, "b1s": b1sh, "b2c": b2ch, "b3c": b3ch,
        }
        for i in range(N_CORES)
    ]


def _gather(core_outs):
    return np.concatenate(
        [np.ascontiguousarray(o["outT"].T) for o in core_outs], axis=0
    ).astype(np.float32, copy=False)


def kernel(x, W1, b1, W2, b2, W3, b3):
    nc = _build()
    in_maps = _make_in_maps(x, W1, b1, W2, b2, W3, b3)
    res = run_bass_kernel_spmd(nc, in_maps, core_ids=list(range(N_CORES))).results
    return _gather(res)



# revision 10
# speedup vs baseline: 1.3919x; 1.3919x over previous
"""Trainium2 Bass kernel for nn_ComplexNN (3-layer MLP, blended tanh act).

  h1 = blend_act(x @ W1 + b1);  blend_act(z) = z>0 ? 0.9z+0.1tanh(z) : 0.5tanh(z)
  h2 = relu(h1 @ W2 + b2)
  out = h2 @ W3 + b3

Data-parallel over 8 NeuronCores: each core takes 4096 rows of x, weights
replicated. Fully fused on-chip; matmuls in bf16 with fp32 PSUM accumulate.

Ingest: x is pre-cast to bf16 and pre-transposed on the host into
xt[p, k*BL + b] = x[b, k*128 + p] per core, so the kernel DMAs contraction-
major tiles straight into SBUF (no DRAM->DRAM cast, no xbar transposes).
Weights are split across the scalar HWDGE and gpsimd SWDGE queues so w1/w2
land before the PE needs them; the sync HWDGE queue carries only x tiles.

blend_act via  blend(z) = 0.5*t + relu(0.9*z - 0.4*t),  t = tanh(z):
  ACT: t = Tanh(ps + b1)          ACT: r = Relu(0.9*v + 0.9*b1)
  DVE: v = ps - (4/9)*t           DVE: h1 = 0.5*t + r
(for z>0: 0.5t + 0.9z - 0.4t = 0.9z + 0.1t; for z<=0: 0.9z <= 0.4t so the
relu clamps to 0 and h1 = 0.5t.)

mm2's relu+bias runs on DVE (tensor_scalar add,max) to keep ACT under the
PE roofline. Chunks are software-pipelined: mm1(c) runs before mm2(c-1) so
the PE never waits on the blend latency of the last h1 tile.
"""

import sys

sys.path.insert(0, "/opt/trn_rl_repo")

import ml_dtypes
import numpy as np

import concourse.bass as bass
import concourse.mybir as mybir
import concourse.tile as tile
from concourse import bacc
from concourse.bass_utils import run_bass_kernel_spmd

N_CORES = 8
B, D, H, H2, C = 32768, 512, 1024, 512, 10
BL = B // N_CORES  # rows per core = 4096
# Small first chunks fill the pipeline fast; small last chunks shorten the
# mm2->mm3->store drain tail.
CHUNKS = [256, 256, 512, 512, 512, 512, 512, 512, 256, 256]
assert sum(CHUNKS) == BL
KD = D // 128      # 4  k-tiles for mm1
KH = H // 128      # 8  k-tiles for mm2 / h-tiles of h1
KH2 = H2 // 128    # 4  k-tiles for mm3 / h2-tiles of h2

F32 = mybir.dt.float32
BF16 = mybir.dt.bfloat16
AF = mybir.ActivationFunctionType
ALU = mybir.AluOpType


def _body(ctx, tc, outs, ins):
    nc = tc.nc
    xt, w1, w2, w3, b1c, b1s, b2c, b3c = ins
    (outT,) = outs

    wpool = ctx.enter_context(tc.tile_pool(name="weights", bufs=1))
    xpool = ctx.enter_context(tc.tile_pool(name="xT", bufs=6 * KD))
    h1pool = ctx.enter_context(tc.tile_pool(name="h1T", bufs=3 * KH))
    h2pool = ctx.enter_context(tc.tile_pool(name="h2T", bufs=3 * KH2))
    tpool = ctx.enter_context(tc.tile_pool(name="tmp", bufs=6))
    opool = ctx.enter_context(tc.tile_pool(name="ostage", bufs=2))
    mmpool = ctx.enter_context(tc.tile_pool(name="mm", bufs=5, space="PSUM"))
    mm2pool = ctx.enter_context(tc.tile_pool(name="mm2", bufs=2, space="PSUM"))
    mm3pool = ctx.enter_context(tc.tile_pool(name="mm3", bufs=1, space="PSUM"))

    # resident weights / biases.  w1/w2 are output-tile-major so each
    # mm1/mm2 output tile depends on one contiguous 512/1024-col block and
    # the PE can start as soon as the first block lands.
    w1s = wpool.tile([128, KH * KD * 128], BF16)  # [p,(i*KD+k)*128+c] = W1[k*128+p, i*128+c]
    w2s = wpool.tile([128, KH2 * KH * 128], BF16) # [p,(j*KH+k)*128+c] = W2[k*128+p, j*128+c]
    w3s = wpool.tile([128, KH2 * C], BF16)        # w3s[p, k*C + c]  = W3[k*128+p, c]
    b1cs = wpool.tile([128, KH], F32)             # b1cs[p, i] = b1[i*128+p]
    b1ss = wpool.tile([128, KH], F32)             # 0.9 * b1
    b2cs = wpool.tile([128, KH2], F32)
    b3cs = wpool.tile([C, 1], F32)                # b3 as per-partition column

    # Weight loads interleaved across the scalar HWDGE and gpsimd SWDGE
    # queues in PE consumption order (w1 i-blocks, then w2 j-blocks); the
    # sync HWDGE queue is reserved for x-tile ingest.
    W1B = KD * 128   # cols per w1 i-block
    W2B = KH * 128   # cols per w2 j-block

    def load_weights():
        nc.scalar.dma_start(out=w1s[:, :W1B], in_=w1[:, :W1B])
        nc.scalar.dma_start(out=b1cs[:], in_=b1c[:])
        nc.scalar.dma_start(out=b1ss[:], in_=b1s[:])
        nc.gpsimd.dma_start(out=w1s[:, W1B : 2 * W1B], in_=w1[:, W1B : 2 * W1B])
        for i in (2, 4, 6):
            nc.scalar.dma_start(
                out=w1s[:, i * W1B : (i + 1) * W1B], in_=w1[:, i * W1B : (i + 1) * W1B]
            )
        for i in (3, 5, 7):
            nc.gpsimd.dma_start(
                out=w1s[:, i * W1B : (i + 1) * W1B], in_=w1[:, i * W1B : (i + 1) * W1B]
            )
        nc.scalar.dma_start(out=w2s[:, :W2B], in_=w2[:, :W2B])
        nc.scalar.dma_start(out=b2cs[:], in_=b2c[:])
        nc.gpsimd.dma_start(out=w2s[:, W2B : 2 * W2B], in_=w2[:, W2B : 2 * W2B])
        nc.scalar.dma_start(out=w2s[:, 2 * W2B : 3 * W2B], in_=w2[:, 2 * W2B : 3 * W2B])
        nc.gpsimd.dma_start(out=w2s[:, 3 * W2B :], in_=w2[:, 3 * W2B :])
        nc.scalar.dma_start(out=w3s[:], in_=w3[:])
        nc.scalar.dma_start(out=b3cs[:], in_=b3c[:])

    def mm1_blend(xT, NB):
        """mm1 + blend_act for one chunk; returns 8 h1T tiles [128, NB]."""
        h1T = []
        for i in range(KH):
            ps = mmpool.tile([128, NB], F32, tag="ps")
            for k in range(KD):
                nc.tensor.matmul(
                    ps[:],
                    w1s[:, (i * KD + k) * 128 : (i * KD + k + 1) * 128],
                    xT[k][:],
                    start=(k == 0),
                    stop=(k == KD - 1),
                )
            t = tpool.tile([128, NB], BF16, tag="t")
            nc.scalar.activation(t[:], ps[:], AF.Tanh, bias=b1cs[:, i : i + 1], scale=1.0)
            v = tpool.tile([128, NB], BF16, tag="v")
            nc.vector.scalar_tensor_tensor(v[:], t[:], -4.0 / 9.0, ps[:], ALU.mult, ALU.add)
            r = tpool.tile([128, NB], BF16, tag="r")
            nc.scalar.activation(r[:], v[:], AF.Relu, bias=b1ss[:, i : i + 1], scale=0.9)
            h1 = h1pool.tile([128, NB], BF16, tag="h1")
            nc.vector.scalar_tensor_tensor(h1[:], t[:], 0.5, r[:], ALU.mult, ALU.add)
            h1T.append(h1)
        return h1T

    def mm2_relu(h1T, NB, rows):
        """mm2 + relu/bias (gpsimd) for one chunk; returns 4 h2T tiles."""
        h2T = []
        for j in range(KH2):
            ps2 = mm2pool.tile([128, NB], F32, tag="ps2")
            for k in range(KH):
                nc.tensor.matmul(
                    ps2[:],
                    w2s[:, (j * KH + k) * 128 : (j * KH + k + 1) * 128],
                    h1T[k][:],
                    start=(k == 0),
                    stop=(k == KH - 1),
                )
            h2 = h2pool.tile([128, NB], BF16, tag="h2")
            nc.vector.tensor_scalar(h2[:], ps2[:], b2cs[:, j : j + 1], 0.0, ALU.add, ALU.max)
            h2T.append(h2)
        return h2T, NB, rows

    def mm3_store(h2T, NB, rows, last=False):
        ps3 = mm3pool.tile([C, NB], F32, tag="ps3")
        for k in range(KH2):
            nc.tensor.matmul(
                ps3[:],
                w3s[:, k * C : (k + 1) * C],
                h2T[k][:],
                start=(k == 0),
                stop=(k == KH2 - 1),
            )
        stage = opool.tile([C, NB], F32, tag="stage")
        nc.vector.tensor_scalar_add(stage[:], ps3[:], b3cs[:])
        # Tail chunks store via the (long idle) sync HWDGE queue so the final
        # drain isn't gated on the gpsimd SWDGE ring.
        eng = nc.sync if last else nc.gpsimd
        eng.dma_start(out=outT[:, rows], in_=stage[:])

    row0 = 0
    p1 = None  # (h1T, NB, rows) awaiting mm2
    p2 = None  # (h2T, NB, rows) awaiting mm3
    for c, NB in enumerate(CHUNKS):
        rows = slice(row0, row0 + NB)
        row0 += NB

        xT = []
        for k in range(KD):
            xtile = xpool.tile([128, max(CHUNKS)], BF16, tag="xt", name="xt")[:, :NB]
            nc.sync.dma_start(out=xtile[:], in_=xt[:, k * BL + rows.start : k * BL + rows.stop])
            xT.append(xtile)
        if c == 0:
            load_weights()

        if p2 is not None:
            mm3_store(*p2)
        h1T = mm1_blend(xT, NB)
        p2 = mm2_relu(*p1) if p1 is not None else None
        p1 = (h1T, NB, rows)
    mm3_store(*p2)
    p2 = mm2_relu(*p1)
    mm3_store(*p2, last=True)


_CACHED = None


def _build():
    global _CACHED
    if _CACHED is not None:
        return _CACHED
    nc = bacc.Bacc(
        "TRN2",
        target_bir_lowering=False,
        debug=False,
        enable_asserts=False,
        num_devices=N_CORES,
    )
    xt = nc.dram_tensor("xt", [128, KD * BL], BF16, kind="ExternalInput").ap()
    w1 = nc.dram_tensor("w1", [128, KD * H], BF16, kind="ExternalInput").ap()
    w2 = nc.dram_tensor("w2", [128, KH * H2], BF16, kind="ExternalInput").ap()
    w3 = nc.dram_tensor("w3", [128, KH2 * C], BF16, kind="ExternalInput").ap()
    b1c = nc.dram_tensor("b1c", [128, KH], F32, kind="ExternalInput").ap()
    b1s = nc.dram_tensor("b1s", [128, KH], F32, kind="ExternalInput").ap()
    b2c = nc.dram_tensor("b2c", [128, KH2], F32, kind="ExternalInput").ap()
    b3c = nc.dram_tensor("b3c", [C, 1], F32, kind="ExternalInput").ap()
    outT = nc.dram_tensor("outT", [C, BL], F32, kind="ExternalOutput").ap()

    from contextlib import ExitStack

    with tile.TileContext(nc) as tc, ExitStack() as ctx:
        _body(ctx, tc, [outT], [xt, w1, w2, w3, b1c, b1s, b2c, b3c])
    nc.compile()
    _CACHED = nc
    return nc


def _prep_weights(W1, b1, W2, b2, W3, b3):
    bf = ml_dtypes.bfloat16
    w1h = np.ascontiguousarray(
        W1.astype(bf).reshape(KD, 128, KH, 128).transpose(1, 2, 0, 3).reshape(128, KD * H)
    )
    w2h = np.ascontiguousarray(
        W2.astype(bf).reshape(KH, 128, KH2, 128).transpose(1, 2, 0, 3).reshape(128, KH * H2)
    )
    w3h = np.ascontiguousarray(
        W3.astype(bf).reshape(KH2, 128, C).transpose(1, 0, 2).reshape(128, KH2 * C)
    )
    b1f = b1.astype(np.float32)
    b1ch = np.ascontiguousarray(b1f.reshape(KH, 128).T)
    b1sh = np.ascontiguousarray((0.9 * b1f).reshape(KH, 128).T)
    b2ch = np.ascontiguousarray(b2.astype(np.float32).reshape(KH2, 128).T)
    b3ch = np.ascontiguousarray(b3.astype(np.float32).reshape(C, 1))
    return w1h, w2h, w3h, b1ch, b1sh, b2ch, b3ch


def _make_in_maps(x, W1, b1, W2, b2, W3, b3):
    bf = ml_dtypes.bfloat16
    x = np.asarray(x, dtype=np.float32)
    # xt[core][p, k*BL + b] = x[core*BL + b, k*128 + p]
    xth = np.ascontiguousarray(
        x.reshape(N_CORES, BL, KD, 128).transpose(0, 3, 2, 1)
    ).astype(bf).reshape(N_CORES, 128, KD * BL)
    w1h, w2h, w3h, b1ch, b1sh, b2ch, b3ch = _prep_weights(
        np.asarray(W1), np.asarray(b1), np.asarray(W2), np.asarray(b2),
        np.asarray(W3), np.asarray(b3),
    )
    return [
        {
            "xt": xth[i],
            "w1": w1h, "w2": w2h, "w3": w3h,
            "b1c": b1ch, "b1s": b1sh, "b2c": b2ch, "b3c": b3ch,
        }
        for i in range(N_CORES)
    ]


def _gather(core_outs):
    return np.concatenate(
        [np.ascontiguousarray(o["outT"].T) for o in core_outs], axis=0
    ).astype(np.float32, copy=False)


def kernel(x, W1, b1, W2, b2, W3, b3):
    nc = _build()
    in_maps = _make_in_maps(x, W1, b1, W2, b2, W3, b3)
    res = run_bass_kernel_spmd(nc, in_maps, core_ids=list(range(N_CORES))).results
    return _gather(res)
